# revision 52
# baseline (speedup 1.0000x reference)
"""Trainium2 Bass kernel for the Hoyer-spike attention module (B=8,N=1024,C=768,H=12).

Math (per batch, per head): xf = spike1(x); [q|k|v] = xf @ qkv_w.T; ks,vs =
spike2(k),spike2(v) (binary); y = q @ (ks.T @ vs) (exact reassociation of
(q@ks.T)@vs -- no softmax); z = spike3(y) with torch's reshape(B,H,D,N)
reinterpretation; out = z @ proj_w.T + proj_b.

Distribution: data-parallel over B=8 -> one batch per NeuronCore, weights
replicated, no collectives.

Numerics: the q/k/v matmuls feed binary spikes, so any relative error eps
in the pre-spike values flips ~eps of the bits and costs ~sqrt(eps) in
final relative error -- the qkv weights need ~1e-5 effective error.  The
PE cost model charges ap_size(out) x cycles/row per PASS, with fp8
DoubleRow contracting TWO 128-chunks per pass at 0.5 cyc/row -- 4x the
contraction throughput of an fp16 pass.  So every qkv pass is fp8 DR:
the weights are split into THREE e4m3 levels (w*G = L0 + L1/64 + L2/64
with G=64 keeping values out of denormals; mid+lo share one x*2^-6
operand so level scales fold host-side).  Three e4m3 roundings leave
~1.4e-5 rms weight error, matching the old fp16+e5m2 scheme at 0.75
instead of 1.25 fp16-pass units (phase 2: 57.6us -> 34.6us of PE).
2-level kv was tested and fails (rel err 0.07).  M = ks.T@vs is exact
integers <=1024; kvs is stored e4m3 so M rides DR passes over nk-block
pairs where legal: the ISA forbids DoubleRow at tile_position col base
64, so M is emitted twice -- DR pairs into the (0,0) quadrant and plain
fp8 per-nk into (0,64) -- which duplicates M on both psum partition
halves and makes m16 two plain Act copies (the old partition-moving
DMA chain serialized ~4us).  The y-matmul packs q-hi/q-lo fp16 into the
two partition halves of one 128-deep contraction against the duplicated
M.  proj runs fp8e4 hi+lo DoubleRow (x1024 row scale, direct error only).
z: even head-pairs emit Sign {-1,+1} on Act, odd emit {0,1} on DVE
(encoding folded into proj row scales + colsum bias) -- measured faster
than any single-engine or Pool-assisted z split.

Schedule (sim-profiled): PE is the bottleneck (~52us busy of ~61us).
DMA trigger costs are first-class: HWDGE (sync/scalar queues) holds a
global device ~630ns per trigger, Pool SWDGE ~1us generation, and every
DMA completion pays 900ns sem propagation.  Hence: x streams as
half-chunks (first halves before second halves) on the Pool/Act queues;
q weight levels land as 3 chunk-pair slabs matched to the tp-outer pass
order; kv weights land as per-kvf column slabs feeding a kvf-outer psum
sweep; q-wave passes run l-middle so six psums consume each slab
back-to-back; proj passes run tp-outer so only the last two passes per
psum wait on the final z chunks; M passes are pipelined into the kvf2
sweep two psums behind their spikes.  Weight DMA triggers go first on
SP so the first wq slab isn't queued behind x triggers (saves ~1.7us of
PE start lag); txA/txT ride one combined DMA.  PSUM: all psum tiles are
full-bank 512-wide f32 (the sim's pending-zero rows are 2KB; narrower
tiles alias partition-64 offsets).  The torch reshape(B,H,D,N) shuffle
is an addressing trick: stride-16 stationary q slices land zT directly;
the proj matmul runs transposed so proj_b is a per-partition Act bias.
The PE p-state ramp (half speed until 3us of CONTINUOUS busy, reset by
any idle) is held warm by dummy matmuls on a memset scratch tile: a
burst at t~0.4us (no DMA dependency) plus short bursts bridging the
known x-DMA stall windows, so real passes run at full clock.  Half-1
spike/xf1 ops are emitted after wave-0's epilogues: engine queues are
FIFO, and emitting them inline would queue them (waiting on late x
pieces) ahead of the epilogues that recycle wave-0's psums.
"""
import sys
sys.path.insert(0, '/opt/trn_rl_repo')
import numpy as np
import ml_dtypes

import concourse.bass as bass
import concourse.mybir as mybir
import concourse.tile as tile
from concourse import bacc

F32 = mybir.dt.float32
FP16 = mybir.dt.float16
FP8 = mybir.dt.float8e4
AOT = mybir.AluOpType
DR = mybir.MatmulPerfMode.DoubleRow
ACT = mybir.ActivationFunctionType

B, N, C, H, D = 8, 1024, 768, 12, 64
EPS, XS = 1e-5, 1.0
NCORES = 8
E4 = np.dtype(ml_dtypes.float8_e4m3)
SP = 1024.0   # proj weight scale; z spike value 1/64 -> psum = SP/64 * out
G = 64.0      # global qkv weight scale (keeps e4m3 levels out of denormals)
SLVL = 64.0   # mid/lo level scale; their x operand is 2^-6 so SLVL*2^-6 = 1
KVL = 3       # e4m3 levels for the k|v weights (q always uses 3)


def build_nc(rounds=1, upto=5):
    nc = bacc.Bacc(None, target_bir_lowering=False)
    xt_d = nc.declare_dram_parameter("xt", [C, N], F32, isOutput=False)
    wq_d = [nc.declare_dram_parameter(f"wq{l}", [128, 6 * C], FP8, isOutput=False)
            for l in range(3)]
    wkv_d = [nc.declare_dram_parameter(f"wkv{l}", [128, 6 * 2 * C], FP8,
                                       isOutput=False) for l in range(KVL)]
    phi_d = nc.declare_dram_parameter("p_hi", [128, 6 * C], FP8, isOutput=False)
    plo_d = nc.declare_dram_parameter("p_lo", [128, 6 * C], FP8, isOutput=False)
    pb_d = nc.declare_dram_parameter("pb", [128, 7], F32, isOutput=False)
    txat_d = nc.declare_dram_parameter("txAT", [128, 12], F32, isOutput=False)
    tkv_d = nc.declare_dram_parameter("tkv", [128, 2 * C], F32, isOutput=False)
    tyt_d = nc.declare_dram_parameter("tyT", [128, 6], F32, isOutput=False)
    tytn_d = nc.declare_dram_parameter("tytn", [128, 6], F32, isOutput=False)
    out_d = nc.declare_dram_parameter("out", [C, N], F32, isOutput=True)

    with tile.TileContext(nc) as tc:
        with (
            tc.tile_pool(name="const", bufs=1) as const,
            tc.tile_pool(name="work", bufs=2) as work,
            tc.tile_pool(name="mm", bufs=8, space="PSUM") as mm,
        ):
            # ---- constants ----
            txat = const.tile([128, 12], F32, name="txat")
            tkv = const.tile([128, 2 * C], F32, name="tkv")
            tyt = const.tile([128, 6], F32, name="tyt")
            tytn = const.tile([128, 6], F32, name="tytn")
            nc.sync.dma_start(txat[:], txat_d[:])

            wq = [const.tile([128, 6 * C], FP8, name=f"wq{l}") for l in range(3)]
            wkv = [const.tile([128, 6 * 2 * C], FP8, name=f"wkv{l}")
                   for l in range(KVL)]
            p_hi = const.tile([128, 6 * C], FP8, name="p_hi")
            p_lo = const.tile([128, 6 * C], FP8, name="p_lo")
            pb = const.tile([128, 7], F32, name="pb")

            phi3 = p_hi[:, :].rearrange("p (t c) -> p t c", t=6)
            plo3 = p_lo[:, :].rearrange("p (t c) -> p t c", t=6)
            wq3 = [w[:, :].rearrange("p (t c) -> p t c", t=6) for w in wq]
            wkv3 = [w[:, :].rearrange("p (t c) -> p t c", t=6) for w in wkv]

            for _r in range(rounds):
                # ---- phase 1: xT -> spikes xf0 (e4m3 {0,1}, DVE) and xf1
                # (e4m3 {0, 2^-6}, Act Identity scale copy; 2^-6 is a normal
                # e4m3 so no denormal/mask pitfalls) ----
                # x streams in half-chunks, all first halves before second
                # halves, so the nf=0 q-wave starts after ~half the x bytes.
                xt = const.tile([128, 6 * N], F32, name=f"xt_{_r}", tag="xt")
                xf0 = const.tile([128, 6 * N], FP8, name=f"xf0_{_r}", tag="xf0")
                xf1 = const.tile([128, 6 * N], FP8, name=f"xf1_{_r}", tag="xf1")
                # q weight levels trigger first on SP so the HWDGE is clear
                # and wq0 lands ~2.5us; x half-0 goes via Pool's SWDGE (no
                # HWDGE contention), half-1 split Pool/Act.
                for tp in range(3):
                    for l in range(3):
                        nc.sync.dma_start(
                            wq[l][:, tp * 2 * C:(tp + 1) * 2 * C],
                            wq_d[l][:, tp * 2 * C:(tp + 1) * 2 * C])
                # half-1's xf1 Act ops are deferred: Act's FIFO would queue
                # them (waiting on late x pieces) ahead of the wave-0 qTh
                # epilogues, stalling psum recycling for wave 1 by ~2.5us
                xf1_deferred = []
                for half in range(2):
                    for ck in range(6):
                        hs = slice(half * 512, (half + 1) * 512)
                        cs = slice(ck * N + half * 512, ck * N + (half + 1) * 512)
                        x_dma = (nc.gpsimd.dma_start if (half == 0 or ck % 2 == 0)
                                 else nc.scalar.dma_start)
                        x_dma(xt[:, cs], xt_d[ck * 128:(ck + 1) * 128, hs])
                        if half == 0:
                            nc.vector.tensor_scalar(xf0[:, cs], xt[:, cs],
                                                    txat[:, ck:ck + 1], txat[:, 6 + ck:7 + ck],
                                                    AOT.mult, AOT.is_ge)
                            nc.scalar.activation(xf1[:, cs], xf0[:, cs],
                                                 ACT.Identity, bias=0.0,
                                                 scale=1.0 / SLVL)
                        else:
                            xf1_deferred.append((ck, cs))
                nc.sync.dma_start(tkv[:], tkv_d[:])
                # kv weights land as per-kvf column slabs (level-inner) so the
                # kvf-outer kv sweep can start after ~1/3 of the kv bytes
                wkv_dram3 = [w[:, :].rearrange("p (t c) -> p t c", t=6)
                             for w in wkv_d]
                for kvf in range(3):
                    for l in range(KVL):
                        nc.sync.dma_start(
                            wkv3[l][:, :, kvf * 512:(kvf + 1) * 512],
                            wkv_dram3[l][:, :, kvf * 512:(kvf + 1) * 512])
                nc.sync.dma_start(tyt[:], tyt_d[:])
                nc.sync.dma_start(tytn[:], tytn_d[:])
                nc.sync.dma_start(pb[:], pb_d[:])
                nc.sync.dma_start(p_hi[:], phi_d[:])
                nc.sync.dma_start(p_lo[:], plo_d[:])
                xf03 = xf0[:, :].rearrange("p (t n) -> p t n", t=6)
                xf13 = xf1[:, :].rearrange("p (t n) -> p t n", t=6)

                if upto < 2:
                    nc.sync.dma_start(out_d[0:128, 0:N], xt[:, 0:N])
                    continue
                # ---- phase 2: qT (shuffled layout) = A_o * (Wq @ xfT).
                # All passes are fp8 DoubleRow (0.5 cyc/row, 2 chunks/pass):
                # 3 weight levels (e4m3 hi/mid/lo, mid+lo share the x*2^-9
                # operand) x 3 chunk-pairs = 9 passes per psum, vs 6 fp16 +
                # 3 DR = 4.9x the rows before. qTh/qTl fp16 hi+lo of the
                # psum preserve ~fp32 q for the y-matmul. ----
                # chunk hp holds heads (2hp, 2hp+1) on partitions 0:64 / 64:128.
                # Shuffled free axis: col m = (n%16)*64 + n//16 so the y-matmul
                # lhsT slices are contiguous.
                qTh = [const.tile([128, N], FP16, name=f"qTh{hp}_{_r}", tag=f"qTh{hp}")
                       for hp in range(6)]
                qTl = [const.tile([128, N], FP16, name=f"qTl{hp}_{_r}", tag=f"qTl{hp}")
                       for hp in range(6)]
                qp = [const.tile([128, N], FP16, name=f"qp{h}_{_r}", tag=f"qp{h}")
                      for h in range(H)]
                kvsa = const.tile([128, 8 * 2 * C], FP8,
                                  name=f"kvsa_{_r}", tag="kvsa")
                kvs4 = kvsa[:, :].rearrange("p (nk c) -> p nk c", nk=8)

                def emit_kv_psum(nk, kvf):
                    p = mm.tile([128, 512], F32, name=f"kvp{nk}_{kvf}_{_r}", tag="mm")
                    for tp in range(3):
                        for l in range(KVL):
                            xs = xf03 if l == 0 else xf13
                            nc.tensor.matmul(
                                p[:],
                                xs[:, 2 * tp:2 * tp + 2, nk * 128:(nk + 1) * 128],
                                wkv3[l][:, 2 * tp:2 * tp + 2, kvf * 512:(kvf + 1) * 512],
                                start=(tp == 0 and l == 0),
                                stop=(tp == 2 and l == KVL - 1), perf_mode=DR)
                    nc.vector.tensor_tensor(
                        kvsa[:, nk * 2 * C + kvf * 512: nk * 2 * C + (kvf + 1) * 512],
                        p[:], tkv[:, kvf * 512:(kvf + 1) * 512], AOT.is_ge)

                # ramp-bridging dummies: the PE p-state clock resets on any
                # idle, and the early x/weight DMA latencies leave 0.4-1.8us
                # stalls that would hold the PE at half speed for 3us after
                # each.  Cheap matmuls into a never-read scratch psum span
                # the known stall windows so the ramp completes once.
                scr = mm.tile([128, 512], F32, name=f"scr_{_r}", tag="mm")
                mset = const.tile([128, 16], F32, name=f"mset_{_r}", tag="mset")
                nc.vector.memset(mset[:], 0.0)

                def dummy_txat(i):
                    # operand is a memset tile: no DMA dependency, so the
                    # ramp clock starts ~0.5us in instead of ~2.4us
                    nc.tensor.matmul(scr[0:16, 0:16], mset[:, 0:16],
                                     mset[:, 0:16], start=True, stop=True,
                                     skip_group_check=True)

                def dummy_wq(i):
                    nc.tensor.matmul(scr[:], wq3[0][:, 0:2, 0:128],
                                     wq3[0][:, 0:2, 0:512], start=True,
                                     stop=True, skip_group_check=True,
                                     perf_mode=DR)

                for i in range(35):
                    dummy_txat(i)

                # q waves by n-half: wave nf uses only that half of x
                for nf in range(2):
                    ps = {}
                    for hp in range(6):
                        ps[hp] = mm.tile([128, 512], F32,
                                         name=f"qp{hp}_{nf}_{_r}", tag="mm")
                    # l middle loop: all six psums consume weight slab
                    # (tp, l) back-to-back, so the PE isn't stalled on slab
                    # l+1's DMA while slab l passes are still runnable
                    ns = slice(nf * 512, (nf + 1) * 512)
                    for tp in range(3):
                        if nf == 0 and tp == 1:
                            for i in range(7):
                                dummy_wq(i)
                        for l in range(3):
                            xs = xf03 if l == 0 else xf13
                            for hp in range(6):
                                nc.tensor.matmul(
                                    ps[hp][:],
                                    wq3[l][:, 2 * tp:2 * tp + 2, hp * 128:(hp + 1) * 128],
                                    xs[:, 2 * tp:2 * tp + 2, nf * 512:(nf + 1) * 512],
                                    start=(tp == 0 and l == 0),
                                    stop=(tp == 2 and l == 2), perf_mode=DR)
                                if tp == 2 and l == 2:
                                    # epilogue emitted right as each psum
                                    # closes so the recycle drain overlaps
                                    # the remaining passes: qTh = fp16(psum),
                                    # qTl = psum - qTh
                                    nc.scalar.activation(qTh[hp][:, ns],
                                                         ps[hp][:, :],
                                                         ACT.Identity,
                                                         bias=0.0, scale=1.0)
                                    nc.vector.tensor_tensor(qTl[hp][:, ns],
                                                            ps[hp][:, :],
                                                            qTh[hp][:, ns],
                                                            AOT.subtract)
                    if nf == 0:
                        for i in range(6):
                            dummy_wq(i)
                    if nf == 0:
                        for ck, cs in xf1_deferred:
                            nc.vector.tensor_scalar(xf0[:, cs], xt[:, cs],
                                                    txat[:, ck:ck + 1], txat[:, 6 + ck:7 + ck],
                                                    AOT.mult, AOT.is_ge)
                            nc.scalar.activation(xf1[:, cs], xf0[:, cs],
                                                 ACT.Identity, bias=0.0,
                                                 scale=1.0 / SLVL)
                for hp in range(6):
                    # assemble per-head packed tiles (hi on one half, lo on
                    # the other) with SBUF->SBUF DMAs on idle queues; P4
                    # consumes qp much later so latency is free
                    q_dma = nc.gpsimd.dma_start if hp % 2 == 0 else nc.sync.dma_start
                    q_dma(qp[2 * hp][0:64, :], qTh[hp][0:64, :])
                    q_dma(qp[2 * hp][64:128, :], qTl[hp][0:64, :])
                    q_dma(qp[2 * hp + 1][64:128, :], qTh[hp][64:128, :])
                    q_dma(qp[2 * hp + 1][0:64, :], qTl[hp][64:128, :])

                if upto < 3:
                    nc.gpsimd.dma_start(out_d[0:128, 0:N], qTh[0][:, 0:N])
                    continue
                # ---- phase 3: k|v psums (kvf-outer so the sweep starts after
                # the first kv weight slab) + spikes + M accumulation ----
                # M is emitted to BOTH psum column-quadrants (heads 0-5 in mA,
                # 6-11 in mB, duplicated on partition halves), so m16 is two
                # plain Act copies -- no partition-moving DMA chain.
                # full-bank width: the sim's psum pending-zero rows are 2KB,
                # so a 384-wide tile would alias partition 64 writes
                mA = mm.tile([128, 512], F32, name=f"mA_{_r}", tag="mm")
                mB = mm.tile([128, 512], F32, name=f"mB_{_r}", tag="mm")

                def emit_m_dr(j):
                    # DR pass contracts nk blocks (2j, 2j+1) into the (0,0)
                    # column quadrant; the ISA forbids DR at col base 64, so
                    # the partition-64 duplicate rides plain fp8 per-nk below.
                    # start/stop once per partition-half per tile: the sim's
                    # psum pending-zero marking is zero-region (bank) wide
                    for h in range(H):
                        T = mA if h < 6 else mB
                        hc = h % 6
                        nc.tensor.matmul(T[0:64, hc * 64:(hc + 1) * 64],
                                         kvs4[:, 2 * j:2 * j + 2, h * 64:(h + 1) * 64],
                                         kvs4[:, 2 * j:2 * j + 2, C + h * 64: C + (h + 1) * 64],
                                         start=(j == 0 and hc == 0),
                                         stop=(j == 3 and hc == 5),
                                         tile_position=(0, 0),
                                         skip_group_check=True,
                                         perf_mode=DR)

                def emit_m_hi(nk):
                    for h in range(H):
                        T = mA if h < 6 else mB
                        hc = h % 6
                        nc.tensor.matmul(T[64:128, hc * 64:(hc + 1) * 64],
                                         kvsa[:, nk * 2 * C + h * 64: nk * 2 * C + (h + 1) * 64],
                                         kvsa[:, nk * 2 * C + C + h * 64: nk * 2 * C + C + (h + 1) * 64],
                                         start=(nk == 0 and hc == 0),
                                         stop=(nk == 7 and hc == 5),
                                         tile_position=(0, 64),
                                         skip_group_check=True)

                # M passes pipelined into the kvf2 sweep: emit each M pass
                # a couple of psums after the kvs spikes it reads, so the PE
                # never waits on the DVE spike stream
                for kvf in range(3):
                    for nk in range(8):
                        emit_kv_psum(nk, kvf)
                        if kvf == 2 and nk >= 3 and nk % 2 == 1:
                            emit_m_dr((nk - 3) // 2)
                        if kvf == 2 and nk >= 2:
                            emit_m_hi(nk - 2)
                emit_m_dr(3)
                for nk in range(6, 8):
                    emit_m_hi(nk)

                if upto < 4:
                    mdump = work.tile([128, 6 * D], F32, name=f"mdump{_r}", tag="mdump")
                    nc.vector.tensor_copy(mdump[:], mA[:, 0:6 * D])
                    nc.sync.dma_start(out_d[0:128, 0:6 * D], mdump[:, :])
                    continue
                # ---- phase 4: y-matmul (fp16 hi+lo) -> spike -> zT (head
                # pairs packed via tile_position quadrants) ----
                m16 = const.tile([128, H * D], FP16, name=f"m16_{_r}", tag="m16")
                # per-head-pair copies so the first zp group starts after a
                # 128-col copy instead of the whole 384
                for hp6 in range(6):
                    src = mA if hp6 < 3 else mB
                    sc = (hp6 % 3) * 128
                    nc.scalar.activation(m16[:, hp6 * 128:(hp6 + 1) * 128],
                                         src[:, sc:sc + 128], ACT.Identity,
                                         bias=0.0, scale=1.0)
                z8 = const.tile([128, 6 * N], FP8, name=f"z8_{_r}", tag="z8")
                z83 = z8[:, :].rearrange("p (t n) -> p t n", t=6)
                for hp in range(6):
                    hA, hB = 2 * hp, 2 * hp + 1
                    for half in range(2):
                        zp = mm.tile([128, 512], F32, name=f"zp{hp}_{half}_{_r}", tag="mm")
                        for q8 in range(8):
                            qb = half * 8 + q8
                            # one matmul per (head, q8) region: contraction
                            # spans all 128 partitions = hi+lo halves of qp
                            # against duplicated M halves; each region is
                            # written exactly once (skip the sim's coarse
                            # zero-region group check; HW-validated construct)
                            qA = qp[hA][:, :].rearrange("p (a b) -> p a b", b=16)[:, :, qb:qb + 1]
                            qB = qp[hB][:, :].rearrange("p (a b) -> p a b", b=16)[:, :, qb:qb + 1]
                            nc.tensor.matmul(zp[0:64, q8 * 64:(q8 + 1) * 64],
                                             qA,
                                             m16[:, hA * 64:(hA + 1) * 64],
                                             start=True, stop=True,
                                             tile_position=(0, 0),
                                             skip_group_check=True)
                            nc.tensor.matmul(zp[64:128, q8 * 64:(q8 + 1) * 64],
                                             qB,
                                             m16[:, hB * 64:(hB + 1) * 64],
                                             start=True, stop=True,
                                             tile_position=(0, 64),
                                             skip_group_check=True)
                        # z encodings per head-pair block: even hp emit
                        # sign in {-1,+1} on the Activation engine, odd hp
                        # emit {0,1} on DVE; the proj weights/bias fold the
                        # difference (even rows at SP/2 plus a colsum/2 bias)
                        if hp % 2 == 0:
                            nc.scalar.activation(
                                z83[:, hp, half * 512:(half + 1) * 512], zp[:],
                                ACT.Sign, bias=tytn[:, hp:hp + 1], scale=1.0)
                        else:
                            nc.vector.tensor_scalar(
                                z83[:, hp, half * 512:(half + 1) * 512], zp[:],
                                tyt[:, hp:hp + 1], None, AOT.is_ge)

                if upto < 5:
                    nc.gpsimd.dma_start(out_d[0:128, 0:N], qTh[0][:, 0:N])
                    continue
                # ---- phase 5 (transposed): outT[cout, n] = (64/SP) * psum
                # + pb[cout]; stationary = proj weights, moving = z8, so the
                # proj bias is a per-partition Activation bias and out ap=512 ----
                for co in range(6):
                    outs = work.tile([128, N], F32, name=f"outs{co}_{_r}", tag="outs")
                    for half in range(2):
                        pp = mm.tile([128, 512], F32, name=f"pp{co}_{half}_{_r}", tag="mm")
                        # tp-outer so only the last two passes wait on the
                        # final z chunks (z pairs complete in tp order)
                        for tp in range(3):
                            for hl, p3 in enumerate((phi3, plo3)):
                                nc.tensor.matmul(
                                    pp[:],
                                    p3[:, 2 * tp:2 * tp + 2, co * 128:(co + 1) * 128],
                                    z83[:, 2 * tp:2 * tp + 2, half * 512:(half + 1) * 512],
                                    start=(tp == 0 and hl == 0),
                                    stop=(tp == 2 and hl == 1),
                                    perf_mode=DR)
                        nc.scalar.activation(outs[:, half * 512:(half + 1) * 512],
                                             pp[:], ACT.Identity,
                                             bias=pb[:, co:co + 1], scale=1.0 / SP)
                        if co == 5:
                            # split the last chunk's writes across both DMA
                            # queues so the final drain halves
                            for qi, q_dma in enumerate((nc.gpsimd.dma_start,
                                                        nc.sync.dma_start)):
                                q_dma(out_d[co * 128:(co + 1) * 128,
                                            half * 512 + qi * 256:half * 512 + (qi + 1) * 256],
                                      outs[:, half * 512 + qi * 256:half * 512 + (qi + 1) * 256])
                        else:
                            o_dma = nc.gpsimd.dma_start if half == 0 else nc.sync.dma_start
                            o_dma(out_d[co * 128:(co + 1) * 128,
                                        half * 512:(half + 1) * 512],
                                  outs[:, half * 512:(half + 1) * 512])

    return nc


def prep_params(inputs):
    """Host-side folding of BN/Hoyer params + weight transposes/splits."""
    d = {k: np.asarray(v, np.float32) for k, v in inputs.items()}

    def fold(p, a):
        s = d[p + '_g'] / np.sqrt(d[p + '_v'] + EPS)
        thr = float(d[a + '_thr'])
        A = s / thr
        Bc = (d[p + '_b'] - d[p + '_m'] * s) / thr
        T2 = XS * d[a + '_run'] - Bc
        return A.astype(np.float32), T2.astype(np.float32)

    A_x, T2_x = fold('n', 'a')
    A_k, T2_k = fold('nk', 'ak')
    A_v, T2_v = fold('nv', 'av')
    A_o, T2_o = fold('no', 'ao')

    Wt = d['qkv_w'].T.copy()                       # [C, 3C]
    colscale = np.concatenate([np.repeat(A_o, D),
                               np.repeat(A_k, D), np.repeat(A_v, D)])
    Wt *= colscale[None, :]

    def lay(Lx):  # [768, cols] -> [128, 6*cols] chunk-major
        c = Lx.shape[1]
        return np.ascontiguousarray(
            Lx.reshape(6, 128, c).transpose(1, 0, 2).reshape(128, 6 * c))

    # 3-level e4m3 split: w*G = L0 + (2^-9)*L1 + (2^-9)*L2 with each level
    # e4m3-rounded; the matmul passes use x (for L0) and x*2^-9 (for L1,L2).
    def split3(W):
        Wg = (W * G).astype(np.float32)
        L0 = Wg.astype(E4)
        r1 = Wg - L0.astype(np.float32)
        L1 = (r1 * SLVL).astype(E4)
        r2 = r1 - L1.astype(np.float32) / SLVL
        L2 = (r2 * SLVL).astype(E4)
        return [lay(L0), lay(L1), lay(L2)]

    wqL = split3(np.ascontiguousarray(Wt[:, 0:C]))
    wkvL = split3(np.ascontiguousarray(Wt[:, C:3 * C]))

    Pt = np.ascontiguousarray(d['proj_w'].T)       # [C, C]
    rows_even = (np.arange(C) // 128) % 2 == 0
    rowscale = np.where(rows_even, SP / 2.0, SP).astype(np.float32)
    Pt8 = Pt * rowscale[:, None]
    p_hi = Pt8.astype(E4)
    p_lo = (Pt8 - p_hi.astype(np.float32)).astype(E4)
    colsum_even = Pt[rows_even, :].sum(axis=0)

    def part6(vec):  # [768] -> [128, 6]; col ck = partition chunk ck
        return np.ascontiguousarray(vec.reshape(6, 128).T)

    def part7(vec):  # part6 + a col of 1/SP (DVE epilogue scale operand)
        return np.ascontiguousarray(np.concatenate(
            [vec.reshape(6, 128).T,
             np.full((128, 1), 1.0 / SP, np.float32)], axis=1))

    kv_levels = {f'wkv{l}': wkvL[l] for l in range(KVL)}
    return dict(
        wq0=wqL[0], wq1=wqL[1], wq2=wqL[2],
        **kv_levels,
        p_hi=lay(p_hi), p_lo=lay(p_lo),
        txAT=np.concatenate([part6(np.repeat(A_x, D)),
                             part6(np.repeat(T2_x, D))], axis=1),
        tkv=np.ascontiguousarray(np.broadcast_to(
            G * np.concatenate([np.repeat(T2_k, D), np.repeat(T2_v, D)]),
            (128, 2 * C))).astype(np.float32),
        tyT=part6(G * np.repeat(T2_o, D)),
        pb=part7(d['proj_b'] + 0.5 * colsum_even),
        tytn=part6(-G * np.repeat(T2_o, D)),
    )


def make_in_maps(inputs):
    shared = prep_params(inputs)
    x = np.asarray(inputs['x'], np.float32)
    return [dict(shared, xt=np.ascontiguousarray(x[c].T)) for c in range(NCORES)]


_CACHE = {}


def _make_executor(nc, n_cores=NCORES):
    """Jitted SPMD executor for the Bass graph (mirrors
    concourse.bass2jax.run_bass_via_pjrt, kept reusable for repeat runs)."""
    import jax
    from jax.sharding import Mesh, PartitionSpec
    from jax.experimental.shard_map import shard_map
    from concourse.bass2jax import (_bass_exec_p, install_neuronx_cc_hook,
                                    partition_id_tensor)
    install_neuronx_cc_hook()
    partition_name = nc.partition_id_tensor.name if nc.partition_id_tensor else None
    in_names, out_names, out_avals, zero_outs = [], [], [], []
    for alloc in nc.m.functions[0].allocations:
        if not isinstance(alloc, mybir.MemoryLocationSet):
            continue
        name = alloc.memorylocations[0].name
        if alloc.kind == "ExternalInput":
            if name != partition_name:
                in_names.append(name)
        elif alloc.kind == "ExternalOutput":
            out_names.append(name)
            shape = tuple(alloc.tensor_shape)
            dtype = mybir.dt.np(alloc.dtype)
            out_avals.append(jax.core.ShapedArray(shape, dtype))
            zero_outs.append(np.zeros(shape, dtype))
    n_params = len(in_names)
    n_outs = len(out_avals)
    all_in_names = list(in_names) + list(out_names)
    if partition_name is not None:
        all_in_names.append(partition_name)

    def _body(*args):
        operands = list(args)
        if partition_name is not None:
            operands.append(partition_id_tensor())
        outs = _bass_exec_p.bind(
            *operands,
            out_avals=tuple(out_avals), in_names=tuple(all_in_names),
            out_names=tuple(out_names), lowering_input_output_aliases=(),
            sim_require_finite=True, sim_require_nnan=True, nc=nc,
        )
        return tuple(outs)

    try:
        devices = jax.devices("axon")[:n_cores]
    except RuntimeError:
        devices = jax.devices()[:n_cores]
    mesh = Mesh(np.asarray(devices), ("core",))
    in_specs = (PartitionSpec("core"),) * (n_params + n_outs)
    out_specs = (PartitionSpec("core"),) * n_outs
    donate = tuple(range(n_params, n_params + n_outs))
    sharded = jax.jit(
        shard_map(_body, mesh=mesh, in_specs=in_specs, out_specs=out_specs,
                  check_rep=False),
        donate_argnums=donate, keep_unused=True,
    )

    def run(in_maps):
        per_core = [[np.asarray(m[n]) for n in in_names] for m in in_maps]
        concat_in = [np.concatenate([per_core[c][i] for c in range(n_cores)], axis=0)
                     for i in range(n_params)]
        concat_zeros = [np.zeros((n_cores * z.shape[0], *z.shape[1:]), z.dtype)
                        for z in zero_outs]
        out_arrs = sharded(*concat_in, *concat_zeros)
        return [
            {name: np.asarray(out_arrs[i]).reshape(n_cores, *out_avals[i].shape)[c]
             for i, name in enumerate(out_names)}
            for c in range(n_cores)
        ], out_arrs

    def run_device_args(concat_in, concat_zeros):
        return sharded(*concat_in, *concat_zeros)

    return run, run_device_args, (in_names, out_names, out_avals, zero_outs, n_params)


def kernel(**inputs) -> np.ndarray:
    if 'exec' not in _CACHE:
        nc = build_nc()
        nc.compile()
        run, run_dev, meta = _make_executor(nc, NCORES)
        _CACHE['exec'] = (nc, run, run_dev, meta)
    nc, run, run_dev, meta = _CACHE['exec']
    in_maps = make_in_maps(inputs)
    results, _ = run(in_maps)
    return np.stack([np.ascontiguousarray(results[c]['out'].T)
                     for c in range(NCORES)]).astype(np.float32)



# revision 53
# speedup vs baseline: 1.0054x; 1.0054x over previous
"""Trainium2 Bass kernel for the Hoyer-spike attention module (B=8,N=1024,C=768,H=12).

Math (per batch, per head): xf = spike1(x); [q|k|v] = xf @ qkv_w.T; ks,vs =
spike2(k),spike2(v) (binary); y = q @ (ks.T @ vs) (exact reassociation of
(q@ks.T)@vs -- no softmax); z = spike3(y) with torch's reshape(B,H,D,N)
reinterpretation; out = z @ proj_w.T + proj_b.

Distribution: data-parallel over B=8 -> one batch per NeuronCore, weights
replicated, no collectives.

Numerics: the q/k/v matmuls feed binary spikes, so any relative error eps
in the pre-spike values flips ~eps of the bits and costs ~sqrt(eps) in
final relative error -- the qkv weights need ~1e-5 effective error.  The
PE cost model charges ap_size(out) x cycles/row per PASS, with fp8
DoubleRow contracting TWO 128-chunks per pass at 0.5 cyc/row -- 4x the
contraction throughput of an fp16 pass.  So every qkv pass is fp8 DR:
the weights are split into THREE e4m3 levels (w*G = L0 + L1/64 + L2/64
with G=64 keeping values out of denormals; mid+lo share one x*2^-6
operand so level scales fold host-side).  Three e4m3 roundings leave
~1.4e-5 rms weight error, matching the old fp16+e5m2 scheme at 0.75
instead of 1.25 fp16-pass units (phase 2: 57.6us -> 34.6us of PE).
2-level kv was tested and fails (rel err 0.07).  M = ks.T@vs is exact
integers <=1024; kvs is stored e4m3 so M rides DR passes over nk-block
pairs where legal: the ISA forbids DoubleRow at tile_position col base
64, so M is emitted twice -- DR pairs into the (0,0) quadrant and plain
fp8 per-nk into (0,64) -- which duplicates M on both psum partition
halves and makes m16 two plain Act copies (the old partition-moving
DMA chain serialized ~4us).  The y-matmul packs q-hi/q-lo fp16 into the
two partition halves of one 128-deep contraction against the duplicated
M.  proj runs fp8e4 hi+lo DoubleRow (x1024 row scale, direct error only).
z: even head-pairs emit Sign {-1,+1} on Act, odd emit {0,1} on DVE
(encoding folded into proj row scales + colsum bias) -- measured faster
than any single-engine or Pool-assisted z split.

Schedule (sim-profiled): PE is the bottleneck (~52us busy of ~61us).
DMA trigger costs are first-class: HWDGE (sync/scalar queues) holds a
global device ~630ns per trigger, Pool SWDGE ~1us generation, and every
DMA completion pays 900ns sem propagation.  Hence: x streams as
half-chunks (first halves before second halves) on the Pool/Act queues;
q weight levels land as 3 chunk-pair slabs matched to the tp-outer pass
order; kv weights land as per-kvf column slabs feeding a kvf-outer psum
sweep; q-wave passes run l-middle so six psums consume each slab
back-to-back; proj passes run tp-outer so only the last two passes per
psum wait on the final z chunks; M passes are pipelined into the kvf2
sweep two psums behind their spikes.  Weight DMA triggers go first on
SP so the first wq slab isn't queued behind x triggers (saves ~1.7us of
PE start lag); txA/txT ride one combined DMA.  PSUM: all psum tiles are
full-bank 512-wide f32 (the sim's pending-zero rows are 2KB; narrower
tiles alias partition-64 offsets).  The torch reshape(B,H,D,N) shuffle
is an addressing trick: stride-16 stationary q slices land zT directly;
the proj matmul runs transposed so proj_b is a per-partition Act bias.
The PE p-state ramp (half speed until 3us of CONTINUOUS busy, reset by
any idle) is held warm by dummy matmuls on a memset scratch tile: a
burst at t~0.4us (no DMA dependency) plus short bursts bridging the
known x-DMA stall windows, so real passes run at full clock.  Half-1
spike/xf1 ops are emitted after wave-0's epilogues: engine queues are
FIFO, and emitting them inline would queue them (waiting on late x
pieces) ahead of the epilogues that recycle wave-0's psums.
"""
import sys
sys.path.insert(0, '/opt/trn_rl_repo')
import numpy as np
import ml_dtypes

import concourse.bass as bass
import concourse.mybir as mybir
import concourse.tile as tile
from concourse import bacc

F32 = mybir.dt.float32
FP16 = mybir.dt.float16
FP8 = mybir.dt.float8e4
AOT = mybir.AluOpType
DR = mybir.MatmulPerfMode.DoubleRow
ACT = mybir.ActivationFunctionType

B, N, C, H, D = 8, 1024, 768, 12, 64
EPS, XS = 1e-5, 1.0
NCORES = 8
E4 = np.dtype(ml_dtypes.float8_e4m3)
SP = 1024.0   # proj weight scale; z spike value 1/64 -> psum = SP/64 * out
G = 64.0      # global qkv weight scale (keeps e4m3 levels out of denormals)
SLVL = 64.0   # mid/lo level scale; their x operand is 2^-6 so SLVL*2^-6 = 1
KVL = 3       # e4m3 levels for the k|v weights (q always uses 3)


def build_nc(rounds=1, upto=5):
    nc = bacc.Bacc(None, target_bir_lowering=False)
    xt_d = nc.declare_dram_parameter("xt", [C, N], F32, isOutput=False)
    wq_d = [nc.declare_dram_parameter(f"wq{l}", [128, 6 * C], FP8, isOutput=False)
            for l in range(3)]
    wkv_d = [nc.declare_dram_parameter(f"wkv{l}", [128, 6 * 2 * C], FP8,
                                       isOutput=False) for l in range(KVL)]
    phi_d = nc.declare_dram_parameter("p_hi", [128, 6 * C], FP8, isOutput=False)
    plo_d = nc.declare_dram_parameter("p_lo", [128, 6 * C], FP8, isOutput=False)
    pb_d = nc.declare_dram_parameter("pb", [128, 7], F32, isOutput=False)
    txat_d = nc.declare_dram_parameter("txAT", [128, 12], F32, isOutput=False)
    tkv_d = nc.declare_dram_parameter("tkv", [128, 2 * C], F32, isOutput=False)
    tyt_d = nc.declare_dram_parameter("tyT", [128, 6], F32, isOutput=False)
    tytn_d = nc.declare_dram_parameter("tytn", [128, 6], F32, isOutput=False)
    out_d = nc.declare_dram_parameter("out", [C, N], F32, isOutput=True)

    with tile.TileContext(nc) as tc:
        with (
            tc.tile_pool(name="const", bufs=1) as const,
            tc.tile_pool(name="work", bufs=2) as work,
            tc.tile_pool(name="mm", bufs=8, space="PSUM") as mm,
        ):
            # ---- constants ----
            txat = const.tile([128, 12], F32, name="txat")
            tkv = const.tile([128, 2 * C], F32, name="tkv")
            tyt = const.tile([128, 6], F32, name="tyt")
            tytn = const.tile([128, 6], F32, name="tytn")
            nc.sync.dma_start(txat[:], txat_d[:])

            wq = [const.tile([128, 6 * C], FP8, name=f"wq{l}") for l in range(3)]
            wkv = [const.tile([128, 6 * 2 * C], FP8, name=f"wkv{l}")
                   for l in range(KVL)]
            p_hi = const.tile([128, 6 * C], FP8, name="p_hi")
            p_lo = const.tile([128, 6 * C], FP8, name="p_lo")
            pb = const.tile([128, 7], F32, name="pb")

            phi3 = p_hi[:, :].rearrange("p (t c) -> p t c", t=6)
            plo3 = p_lo[:, :].rearrange("p (t c) -> p t c", t=6)
            wq3 = [w[:, :].rearrange("p (t c) -> p t c", t=6) for w in wq]
            wkv3 = [w[:, :].rearrange("p (t c) -> p t c", t=6) for w in wkv]

            for _r in range(rounds):
                # ---- phase 1: xT -> spikes xf0 (e4m3 {0,1}, DVE) and xf1
                # (e4m3 {0, 2^-6}, Act Identity scale copy; 2^-6 is a normal
                # e4m3 so no denormal/mask pitfalls) ----
                # x streams in half-chunks, all first halves before second
                # halves, so the nf=0 q-wave starts after ~half the x bytes.
                xt = const.tile([128, 6 * N], F32, name=f"xt_{_r}", tag="xt")
                xf0 = const.tile([128, 6 * N], FP8, name=f"xf0_{_r}", tag="xf0")
                xf1 = const.tile([128, 6 * N], FP8, name=f"xf1_{_r}", tag="xf1")
                # q weight levels trigger first on SP so the HWDGE is clear
                # and wq0 lands ~2.5us; x half-0 goes via Pool's SWDGE (no
                # HWDGE contention), half-1 split Pool/Act.
                for tp in range(3):
                    for l in range(3):
                        nc.sync.dma_start(
                            wq[l][:, tp * 2 * C:(tp + 1) * 2 * C],
                            wq_d[l][:, tp * 2 * C:(tp + 1) * 2 * C])
                # half-1's xf1 Act ops are deferred: Act's FIFO would queue
                # them (waiting on late x pieces) ahead of the wave-0 qTh
                # epilogues, stalling psum recycling for wave 1 by ~2.5us
                xf1_deferred = []
                for half in range(2):
                    for ck in range(6):
                        hs = slice(half * 512, (half + 1) * 512)
                        cs = slice(ck * N + half * 512, ck * N + (half + 1) * 512)
                        x_dma = (nc.gpsimd.dma_start if (half == 0 or ck % 2 == 0)
                                 else nc.scalar.dma_start)
                        x_dma(xt[:, cs], xt_d[ck * 128:(ck + 1) * 128, hs])
                        if half == 0:
                            nc.vector.tensor_scalar(xf0[:, cs], xt[:, cs],
                                                    txat[:, ck:ck + 1], txat[:, 6 + ck:7 + ck],
                                                    AOT.mult, AOT.is_ge)
                            nc.scalar.activation(xf1[:, cs], xf0[:, cs],
                                                 ACT.Identity, bias=0.0,
                                                 scale=1.0 / SLVL)
                        else:
                            xf1_deferred.append((ck, cs))
                nc.sync.dma_start(tkv[:], tkv_d[:])
                # kv weights land as per-kvf column slabs (level-inner) so the
                # kvf-outer kv sweep can start after ~1/3 of the kv bytes
                wkv_dram3 = [w[:, :].rearrange("p (t c) -> p t c", t=6)
                             for w in wkv_d]
                for kvf in range(3):
                    for l in range(KVL):
                        nc.sync.dma_start(
                            wkv3[l][:, :, kvf * 512:(kvf + 1) * 512],
                            wkv_dram3[l][:, :, kvf * 512:(kvf + 1) * 512])
                nc.sync.dma_start(tyt[:], tyt_d[:])
                nc.sync.dma_start(tytn[:], tytn_d[:])
                nc.sync.dma_start(pb[:], pb_d[:])
                nc.sync.dma_start(p_hi[:], phi_d[:])
                nc.sync.dma_start(p_lo[:], plo_d[:])
                xf03 = xf0[:, :].rearrange("p (t n) -> p t n", t=6)
                xf13 = xf1[:, :].rearrange("p (t n) -> p t n", t=6)

                if upto < 2:
                    nc.sync.dma_start(out_d[0:128, 0:N], xt[:, 0:N])
                    continue
                # ---- phase 2: qT (shuffled layout) = A_o * (Wq @ xfT).
                # All passes are fp8 DoubleRow (0.5 cyc/row, 2 chunks/pass):
                # 3 weight levels (e4m3 hi/mid/lo, mid+lo share the x*2^-9
                # operand) x 3 chunk-pairs = 9 passes per psum, vs 6 fp16 +
                # 3 DR = 4.9x the rows before. qTh/qTl fp16 hi+lo of the
                # psum preserve ~fp32 q for the y-matmul. ----
                # chunk hp holds heads (2hp, 2hp+1) on partitions 0:64 / 64:128.
                # Shuffled free axis: col m = (n%16)*64 + n//16 so the y-matmul
                # lhsT slices are contiguous.
                qTh = [const.tile([128, N], FP16, name=f"qTh{hp}_{_r}", tag=f"qTh{hp}")
                       for hp in range(6)]
                qTl = [const.tile([128, N], FP16, name=f"qTl{hp}_{_r}", tag=f"qTl{hp}")
                       for hp in range(6)]
                qp = [const.tile([128, N], FP16, name=f"qp{h}_{_r}", tag=f"qp{h}")
                      for h in range(H)]
                kvsa = const.tile([128, 8 * 2 * C], FP8,
                                  name=f"kvsa_{_r}", tag="kvsa")
                kvs4 = kvsa[:, :].rearrange("p (nk c) -> p nk c", nk=8)

                def emit_kv_psum(nk, kvf):
                    p = mm.tile([128, 512], F32, name=f"kvp{nk}_{kvf}_{_r}", tag="mm")
                    for tp in range(3):
                        for l in range(KVL):
                            xs = xf03 if l == 0 else xf13
                            nc.tensor.matmul(
                                p[:],
                                xs[:, 2 * tp:2 * tp + 2, nk * 128:(nk + 1) * 128],
                                wkv3[l][:, 2 * tp:2 * tp + 2, kvf * 512:(kvf + 1) * 512],
                                start=(tp == 0 and l == 0),
                                stop=(tp == 2 and l == KVL - 1), perf_mode=DR)
                    nc.vector.tensor_tensor(
                        kvsa[:, nk * 2 * C + kvf * 512: nk * 2 * C + (kvf + 1) * 512],
                        p[:], tkv[:, kvf * 512:(kvf + 1) * 512], AOT.is_ge)

                # ramp-bridging dummies: the PE p-state clock resets on any
                # idle, and the early x/weight DMA latencies leave 0.4-1.8us
                # stalls that would hold the PE at half speed for 3us after
                # each.  Cheap matmuls into a never-read scratch psum span
                # the known stall windows so the ramp completes once.
                scr = mm.tile([128, 512], F32, name=f"scr_{_r}", tag="mm")
                mset = const.tile([128, 16], F32, name=f"mset_{_r}", tag="mset")
                nc.vector.memset(mset[:], 0.0)

                def dummy_txat(i):
                    # operand is a memset tile: no DMA dependency, so the
                    # ramp clock starts ~0.5us in instead of ~2.4us
                    nc.tensor.matmul(scr[0:16, 0:16], mset[:, 0:16],
                                     mset[:, 0:16], start=True, stop=True,
                                     skip_group_check=True)

                def dummy_wq(i):
                    nc.tensor.matmul(scr[:], wq3[0][:, 0:2, 0:128],
                                     wq3[0][:, 0:2, 0:512], start=True,
                                     stop=True, skip_group_check=True,
                                     perf_mode=DR)

                for i in range(35):
                    dummy_txat(i)

                # q waves by n-half: wave nf uses only that half of x
                for nf in range(2):
                    ps = {}
                    for hp in range(6):
                        ps[hp] = mm.tile([128, 512], F32,
                                         name=f"qp{hp}_{nf}_{_r}", tag="mm")
                    # l middle loop: all six psums consume weight slab
                    # (tp, l) back-to-back, so the PE isn't stalled on slab
                    # l+1's DMA while slab l passes are still runnable
                    ns = slice(nf * 512, (nf + 1) * 512)
                    for tp in range(3):
                        if nf == 0 and tp == 1:
                            for i in range(7):
                                dummy_wq(i)
                        for l in range(3):
                            xs = xf03 if l == 0 else xf13
                            for hp in range(6):
                                nc.tensor.matmul(
                                    ps[hp][:],
                                    wq3[l][:, 2 * tp:2 * tp + 2, hp * 128:(hp + 1) * 128],
                                    xs[:, 2 * tp:2 * tp + 2, nf * 512:(nf + 1) * 512],
                                    start=(tp == 0 and l == 0),
                                    stop=(tp == 2 and l == 2), perf_mode=DR)
                                if tp == 2 and l == 2:
                                    # epilogue emitted right as each psum
                                    # closes so the recycle drain overlaps
                                    # the remaining passes: qTh = fp16(psum),
                                    # qTl = psum - qTh
                                    nc.scalar.activation(qTh[hp][:, ns],
                                                         ps[hp][:, :],
                                                         ACT.Identity,
                                                         bias=0.0, scale=1.0)
                                    nc.vector.tensor_tensor(qTl[hp][:, ns],
                                                            ps[hp][:, :],
                                                            qTh[hp][:, ns],
                                                            AOT.subtract)
                    if nf == 0:
                        for i in range(6):
                            dummy_wq(i)
                    if nf == 0:
                        for ck, cs in xf1_deferred:
                            nc.vector.tensor_scalar(xf0[:, cs], xt[:, cs],
                                                    txat[:, ck:ck + 1], txat[:, 6 + ck:7 + ck],
                                                    AOT.mult, AOT.is_ge)
                            nc.scalar.activation(xf1[:, cs], xf0[:, cs],
                                                 ACT.Identity, bias=0.0,
                                                 scale=1.0 / SLVL)
                for hp in range(6):
                    # assemble per-head packed tiles (hi on one half, lo on
                    # the other) with SBUF->SBUF DMAs on idle queues; P4
                    # consumes qp much later so latency is free
                    q_dma = nc.gpsimd.dma_start if hp % 2 == 0 else nc.sync.dma_start
                    q_dma(qp[2 * hp][0:64, :], qTh[hp][0:64, :])
                    q_dma(qp[2 * hp][64:128, :], qTl[hp][0:64, :])
                    q_dma(qp[2 * hp + 1][64:128, :], qTh[hp][64:128, :])
                    q_dma(qp[2 * hp + 1][0:64, :], qTl[hp][64:128, :])

                if upto < 3:
                    nc.gpsimd.dma_start(out_d[0:128, 0:N], qTh[0][:, 0:N])
                    continue
                # ---- phase 3: k|v psums (kvf-outer so the sweep starts after
                # the first kv weight slab) + spikes + M accumulation ----
                # M is emitted to BOTH psum column-quadrants (heads 0-5 in mA,
                # 6-11 in mB, duplicated on partition halves), so m16 is two
                # plain Act copies -- no partition-moving DMA chain.
                # full-bank width: the sim's psum pending-zero rows are 2KB,
                # so a 384-wide tile would alias partition 64 writes
                mA = mm.tile([128, 512], F32, name=f"mA_{_r}", tag="mm")
                mB = mm.tile([128, 512], F32, name=f"mB_{_r}", tag="mm")

                def emit_m_dr(j):
                    # DR pass contracts nk blocks (2j, 2j+1) into the (0,0)
                    # column quadrant; the ISA forbids DR at col base 64, so
                    # the partition-64 duplicate rides plain fp8 per-nk below.
                    # start/stop once per partition-half per tile: the sim's
                    # psum pending-zero marking is zero-region (bank) wide
                    for h in range(H):
                        T = mA if h < 6 else mB
                        hc = h % 6
                        nc.tensor.matmul(T[0:64, hc * 64:(hc + 1) * 64],
                                         kvs4[:, 2 * j:2 * j + 2, h * 64:(h + 1) * 64],
                                         kvs4[:, 2 * j:2 * j + 2, C + h * 64: C + (h + 1) * 64],
                                         start=(j == 0 and hc == 0),
                                         stop=(j == 3 and hc == 5),
                                         tile_position=(0, 0),
                                         skip_group_check=True,
                                         perf_mode=DR)

                def emit_m_hi(nk):
                    for h in range(H):
                        T = mA if h < 6 else mB
                        hc = h % 6
                        nc.tensor.matmul(T[64:128, hc * 64:(hc + 1) * 64],
                                         kvsa[:, nk * 2 * C + h * 64: nk * 2 * C + (h + 1) * 64],
                                         kvsa[:, nk * 2 * C + C + h * 64: nk * 2 * C + C + (h + 1) * 64],
                                         start=(nk == 0 and hc == 0),
                                         stop=(nk == 7 and hc == 5),
                                         tile_position=(0, 64),
                                         skip_group_check=True)

                # M passes pipelined into the kvf2 sweep: emit each M pass
                # a couple of psums after the kvs spikes it reads, so the PE
                # never waits on the DVE spike stream
                for kvf in range(3):
                    for nk in range(8):
                        emit_kv_psum(nk, kvf)
                        if kvf == 2 and nk >= 3 and nk % 2 == 1:
                            emit_m_dr((nk - 3) // 2)
                        if kvf == 2 and nk >= 2:
                            emit_m_hi(nk - 2)
                # m_hi(6) first: its spikes are long done, giving the PE
                # ready work while the nk7 spike (which m_dr(3) and m_hi(7)
                # need) drains off the DVE
                emit_m_hi(6)
                emit_m_dr(3)
                emit_m_hi(7)

                if upto < 4:
                    mdump = work.tile([128, 6 * D], F32, name=f"mdump{_r}", tag="mdump")
                    nc.vector.tensor_copy(mdump[:], mA[:, 0:6 * D])
                    nc.sync.dma_start(out_d[0:128, 0:6 * D], mdump[:, :])
                    continue
                # ---- phase 4: y-matmul (fp16 hi+lo) -> spike -> zT (head
                # pairs packed via tile_position quadrants) ----
                m16 = const.tile([128, H * D], FP16, name=f"m16_{_r}", tag="m16")
                # per-head-pair copies so the first zp group starts after a
                # 128-col copy instead of the whole 384
                for hp6 in range(6):
                    src = mA if hp6 < 3 else mB
                    sc = (hp6 % 3) * 128
                    nc.scalar.activation(m16[:, hp6 * 128:(hp6 + 1) * 128],
                                         src[:, sc:sc + 128], ACT.Identity,
                                         bias=0.0, scale=1.0)
                z8 = const.tile([128, 6 * N], FP8, name=f"z8_{_r}", tag="z8")
                z83 = z8[:, :].rearrange("p (t n) -> p t n", t=6)
                for hp in range(6):
                    hA, hB = 2 * hp, 2 * hp + 1
                    for half in range(2):
                        zp = mm.tile([128, 512], F32, name=f"zp{hp}_{half}_{_r}", tag="mm")
                        for q8 in range(8):
                            qb = half * 8 + q8
                            # one matmul per (head, q8) region: contraction
                            # spans all 128 partitions = hi+lo halves of qp
                            # against duplicated M halves; each region is
                            # written exactly once (skip the sim's coarse
                            # zero-region group check; HW-validated construct)
                            qA = qp[hA][:, :].rearrange("p (a b) -> p a b", b=16)[:, :, qb:qb + 1]
                            qB = qp[hB][:, :].rearrange("p (a b) -> p a b", b=16)[:, :, qb:qb + 1]
                            nc.tensor.matmul(zp[0:64, q8 * 64:(q8 + 1) * 64],
                                             qA,
                                             m16[:, hA * 64:(hA + 1) * 64],
                                             start=True, stop=True,
                                             tile_position=(0, 0),
                                             skip_group_check=True)
                            nc.tensor.matmul(zp[64:128, q8 * 64:(q8 + 1) * 64],
                                             qB,
                                             m16[:, hB * 64:(hB + 1) * 64],
                                             start=True, stop=True,
                                             tile_position=(0, 64),
                                             skip_group_check=True)
                        # z encodings per head-pair block: even hp emit
                        # sign in {-1,+1} on the Activation engine, odd hp
                        # emit {0,1} on DVE; the proj weights/bias fold the
                        # difference (even rows at SP/2 plus a colsum/2 bias)
                        if hp % 2 == 0:
                            nc.scalar.activation(
                                z83[:, hp, half * 512:(half + 1) * 512], zp[:],
                                ACT.Sign, bias=tytn[:, hp:hp + 1], scale=1.0)
                        else:
                            nc.vector.tensor_scalar(
                                z83[:, hp, half * 512:(half + 1) * 512], zp[:],
                                tyt[:, hp:hp + 1], None, AOT.is_ge)

                if upto < 5:
                    nc.gpsimd.dma_start(out_d[0:128, 0:N], qTh[0][:, 0:N])
                    continue
                # ---- phase 5 (transposed): outT[cout, n] = (64/SP) * psum
                # + pb[cout]; stationary = proj weights, moving = z8, so the
                # proj bias is a per-partition Activation bias and out ap=512 ----
                for co in range(6):
                    outs = work.tile([128, N], F32, name=f"outs{co}_{_r}", tag="outs")
                    for half in range(2):
                        pp = mm.tile([128, 512], F32, name=f"pp{co}_{half}_{_r}", tag="mm")
                        # tp-outer so only the last two passes wait on the
                        # final z chunks (z pairs complete in tp order)
                        for tp in range(3):
                            for hl, p3 in enumerate((phi3, plo3)):
                                nc.tensor.matmul(
                                    pp[:],
                                    p3[:, 2 * tp:2 * tp + 2, co * 128:(co + 1) * 128],
                                    z83[:, 2 * tp:2 * tp + 2, half * 512:(half + 1) * 512],
                                    start=(tp == 0 and hl == 0),
                                    stop=(tp == 2 and hl == 1),
                                    perf_mode=DR)
                        nc.scalar.activation(outs[:, half * 512:(half + 1) * 512],
                                             pp[:], ACT.Identity,
                                             bias=pb[:, co:co + 1], scale=1.0 / SP)
                        if co == 5:
                            # split the last chunk's writes across both DMA
                            # queues so the final drain halves
                            for qi, q_dma in enumerate((nc.gpsimd.dma_start,
                                                        nc.sync.dma_start)):
                                q_dma(out_d[co * 128:(co + 1) * 128,
                                            half * 512 + qi * 256:half * 512 + (qi + 1) * 256],
                                      outs[:, half * 512 + qi * 256:half * 512 + (qi + 1) * 256])
                        else:
                            o_dma = nc.gpsimd.dma_start if half == 0 else nc.sync.dma_start
                            o_dma(out_d[co * 128:(co + 1) * 128,
                                        half * 512:(half + 1) * 512],
                                  outs[:, half * 512:(half + 1) * 512])

    return nc


def prep_params(inputs):
    """Host-side folding of BN/Hoyer params + weight transposes/splits."""
    d = {k: np.asarray(v, np.float32) for k, v in inputs.items()}

    def fold(p, a):
        s = d[p + '_g'] / np.sqrt(d[p + '_v'] + EPS)
        thr = float(d[a + '_thr'])
        A = s / thr
        Bc = (d[p + '_b'] - d[p + '_m'] * s) / thr
        T2 = XS * d[a + '_run'] - Bc
        return A.astype(np.float32), T2.astype(np.float32)

    A_x, T2_x = fold('n', 'a')
    A_k, T2_k = fold('nk', 'ak')
    A_v, T2_v = fold('nv', 'av')
    A_o, T2_o = fold('no', 'ao')

    Wt = d['qkv_w'].T.copy()                       # [C, 3C]
    colscale = np.concatenate([np.repeat(A_o, D),
                               np.repeat(A_k, D), np.repeat(A_v, D)])
    Wt *= colscale[None, :]

    def lay(Lx):  # [768, cols] -> [128, 6*cols] chunk-major
        c = Lx.shape[1]
        return np.ascontiguousarray(
            Lx.reshape(6, 128, c).transpose(1, 0, 2).reshape(128, 6 * c))

    # 3-level e4m3 split: w*G = L0 + (2^-9)*L1 + (2^-9)*L2 with each level
    # e4m3-rounded; the matmul passes use x (for L0) and x*2^-9 (for L1,L2).
    def split3(W):
        Wg = (W * G).astype(np.float32)
        L0 = Wg.astype(E4)
        r1 = Wg - L0.astype(np.float32)
        L1 = (r1 * SLVL).astype(E4)
        r2 = r1 - L1.astype(np.float32) / SLVL
        L2 = (r2 * SLVL).astype(E4)
        return [lay(L0), lay(L1), lay(L2)]

    wqL = split3(np.ascontiguousarray(Wt[:, 0:C]))
    wkvL = split3(np.ascontiguousarray(Wt[:, C:3 * C]))

    Pt = np.ascontiguousarray(d['proj_w'].T)       # [C, C]
    rows_even = (np.arange(C) // 128) % 2 == 0
    rowscale = np.where(rows_even, SP / 2.0, SP).astype(np.float32)
    Pt8 = Pt * rowscale[:, None]
    p_hi = Pt8.astype(E4)
    p_lo = (Pt8 - p_hi.astype(np.float32)).astype(E4)
    colsum_even = Pt[rows_even, :].sum(axis=0)

    def part6(vec):  # [768] -> [128, 6]; col ck = partition chunk ck
        return np.ascontiguousarray(vec.reshape(6, 128).T)

    def part7(vec):  # part6 + a col of 1/SP (DVE epilogue scale operand)
        return np.ascontiguousarray(np.concatenate(
            [vec.reshape(6, 128).T,
             np.full((128, 1), 1.0 / SP, np.float32)], axis=1))

    kv_levels = {f'wkv{l}': wkvL[l] for l in range(KVL)}
    return dict(
        wq0=wqL[0], wq1=wqL[1], wq2=wqL[2],
        **kv_levels,
        p_hi=lay(p_hi), p_lo=lay(p_lo),
        txAT=np.concatenate([part6(np.repeat(A_x, D)),
                             part6(np.repeat(T2_x, D))], axis=1),
        tkv=np.ascontiguousarray(np.broadcast_to(
            G * np.concatenate([np.repeat(T2_k, D), np.repeat(T2_v, D)]),
            (128, 2 * C))).astype(np.float32),
        tyT=part6(G * np.repeat(T2_o, D)),
        pb=part7(d['proj_b'] + 0.5 * colsum_even),
        tytn=part6(-G * np.repeat(T2_o, D)),
    )


def make_in_maps(inputs):
    shared = prep_params(inputs)
    x = np.asarray(inputs['x'], np.float32)
    return [dict(shared, xt=np.ascontiguousarray(x[c].T)) for c in range(NCORES)]


_CACHE = {}


def _make_executor(nc, n_cores=NCORES):
    """Jitted SPMD executor for the Bass graph (mirrors
    concourse.bass2jax.run_bass_via_pjrt, kept reusable for repeat runs)."""
    import jax
    from jax.sharding import Mesh, PartitionSpec
    from jax.experimental.shard_map import shard_map
    from concourse.bass2jax import (_bass_exec_p, install_neuronx_cc_hook,
                                    partition_id_tensor)
    install_neuronx_cc_hook()
    partition_name = nc.partition_id_tensor.name if nc.partition_id_tensor else None
    in_names, out_names, out_avals, zero_outs = [], [], [], []
    for alloc in nc.m.functions[0].allocations:
        if not isinstance(alloc, mybir.MemoryLocationSet):
            continue
        name = alloc.memorylocations[0].name
        if alloc.kind == "ExternalInput":
            if name != partition_name:
                in_names.append(name)
        elif alloc.kind == "ExternalOutput":
            out_names.append(name)
            shape = tuple(alloc.tensor_shape)
            dtype = mybir.dt.np(alloc.dtype)
            out_avals.append(jax.core.ShapedArray(shape, dtype))
            zero_outs.append(np.zeros(shape, dtype))
    n_params = len(in_names)
    n_outs = len(out_avals)
    all_in_names = list(in_names) + list(out_names)
    if partition_name is not None:
        all_in_names.append(partition_name)

    def _body(*args):
        operands = list(args)
        if partition_name is not None:
            operands.append(partition_id_tensor())
        outs = _bass_exec_p.bind(
            *operands,
            out_avals=tuple(out_avals), in_names=tuple(all_in_names),
            out_names=tuple(out_names), lowering_input_output_aliases=(),
            sim_require_finite=True, sim_require_nnan=True, nc=nc,
        )
        return tuple(outs)

    try:
        devices = jax.devices("axon")[:n_cores]
    except RuntimeError:
        devices = jax.devices()[:n_cores]
    mesh = Mesh(np.asarray(devices), ("core",))
    in_specs = (PartitionSpec("core"),) * (n_params + n_outs)
    out_specs = (PartitionSpec("core"),) * n_outs
    donate = tuple(range(n_params, n_params + n_outs))
    sharded = jax.jit(
        shard_map(_body, mesh=mesh, in_specs=in_specs, out_specs=out_specs,
                  check_rep=False),
        donate_argnums=donate, keep_unused=True,
    )

    def run(in_maps):
        per_core = [[np.asarray(m[n]) for n in in_names] for m in in_maps]
        concat_in = [np.concatenate([per_core[c][i] for c in range(n_cores)], axis=0)
                     for i in range(n_params)]
        concat_zeros = [np.zeros((n_cores * z.shape[0], *z.shape[1:]), z.dtype)
                        for z in zero_outs]
        out_arrs = sharded(*concat_in, *concat_zeros)
        return [
            {name: np.asarray(out_arrs[i]).reshape(n_cores, *out_avals[i].shape)[c]
             for i, name in enumerate(out_names)}
            for c in range(n_cores)
        ], out_arrs

    def run_device_args(concat_in, concat_zeros):
        return sharded(*concat_in, *concat_zeros)

    return run, run_device_args, (in_names, out_names, out_avals, zero_outs, n_params)


def kernel(**inputs) -> np.ndarray:
    if 'exec' not in _CACHE:
        nc = build_nc()
        nc.compile()
        run, run_dev, meta = _make_executor(nc, NCORES)
        _CACHE['exec'] = (nc, run, run_dev, meta)
    nc, run, run_dev, meta = _CACHE['exec']
    in_maps = make_in_maps(inputs)
    results, _ = run(in_maps)
    return np.stack([np.ascontiguousarray(results[c]['out'].T)
                     for c in range(NCORES)]).astype(np.float32)

